# revision 1
# baseline (speedup 1.0000x reference)
"""Trainium2 Bass kernel for CustomMultiHeadAttention.

B=2, S=2048, D_MODEL=1024, N_HEADS=16, D_HEAD=64, fp32 reference.

Sharding: 8 cores, each core owns 2 heads x both batches (tensor parallel
on heads).  Per core:
  - project Q,K,V for its 2 heads (contraction over full d_model),
  - attention for its 4 (batch, head) pairs,
  - partial output projection (its 128 head-dims x full 1024 out-dims),
  - host sums the 8 partials and adds b_o.

Math formulation (per (b,h) pair):
  scoresT[k,q] = (K_h Q_h^T)          (scale 1/8 folded into w_q,b_q)
  expT = exp(scoresT) * ecb[k,q]      ecb = exp(position_bias) * mask  (host)
  ctxT_aug[0:64, q] = V^T expT        via matmul with V augmented by a
  ctxT_aug[64,   q] = sum_k expT      ones column (free softmax denom)
  ctxT = ctxT_aug[0:64] / ctxT_aug[64]  (K=1-matmul broadcast + DVE mul)
  out_partial = ctxT^T @ w_o[:, hd]^T   (lhsT = ctxT directly)

All matmul operands fp16 (1 cycle/row on PE vs 4 for fp32), accumulation
fp32 in PSUM.
"""

import os
import sys
import numpy as np

for _p in ("/opt/trn_rl_repo", "/root/.axon_site/_ro/trn_rl_repo"):
    if os.path.isdir(_p) and _p not in sys.path:
        sys.path.insert(0, _p)

import concourse.bass as bass
import concourse.mybir as mybir
import concourse.tile as tile
from concourse.bass_utils import run_bass_kernel_spmd

F16 = mybir.dt.float16
F32 = mybir.dt.float32
AF = mybir.ActivationFunctionType

B, S, DM, NH, DH = 2, 2048, 1024, 16, 64
NCORES = 8
HPC = NH // NCORES          # heads per core = 2
HD = HPC * DH               # head-dims per core = 128
NDC = DM // 128             # d_model chunks = 8
NKC = S // 128              # key chunks = 16
NQC = 2                     # q chunks of 1024
QW = S // NQC               # 1024
NSB = S // 128              # s blocks for out proj = 16


def _split_multiwait_dmas(nc):
    """This walrus limits every TPB instruction to ONE sync wait (the ISA
    EVENTS struct has a single wait slot).  Tile emits multi-wait
    instructions (WAR vs readers + WAW vs previous slot writer, and big
    fan-in at pool-release barriers).  Peel all but the last wait off onto
    standalone EventSemaphore instructions executed by the same engine
    sequencer immediately before the instruction."""
    n_split = 0
    for f in nc.m.functions:
        for blk in f.blocks:
            insts = blk.instructions
            if not any(i.sync_info and len(i.sync_info.on_wait) > 1
                       for i in insts):
                continue
            out = []
            for inst in insts:
                si = inst.sync_info
                if si is not None and len(si.on_wait) > 1:
                    waits = list(si.on_wait)
                    for j, w in enumerate(waits[:-1]):
                        es = mybir.InstEventSemaphore(name=f"{inst.name}-w{j}")
                        es.engine = inst.engine
                        es.sync_info = mybir.SyncInfo(on_wait=[w], on_update=[])
                        out.append(es)
                        n_split += 1
                    inst.sync_info = mybir.SyncInfo(
                        on_wait=[waits[-1]], on_update=list(si.on_update))
                out.append(inst)
            blk.instructions = out
    return n_split


def _build_program():
    nc = bass.Bass("TRN2", debug=False)

    xq = nc.dram_tensor("xq", [B, DM, S], F16, kind="ExternalInput")
    xk = nc.dram_tensor("xk", [B, DM, S], F16, kind="ExternalInput")
    xv = nc.dram_tensor("xv", [B, DM, S], F16, kind="ExternalInput")
    wq = nc.dram_tensor("wq", [NDC, 128, HD], F16, kind="ExternalInput")
    wk = nc.dram_tensor("wk", [NDC, 128, HD], F16, kind="ExternalInput")
    wv = nc.dram_tensor("wv", [NDC, 128, HD], F16, kind="ExternalInput")
    bq = nc.dram_tensor("bq", [HD, 1], F32, kind="ExternalInput")
    bk = nc.dram_tensor("bk", [HD, 1], F32, kind="ExternalInput")
    bv = nc.dram_tensor("bv", [HD, 1], F32, kind="ExternalInput")
    wo = nc.dram_tensor("wo", [HD, DM], F16, kind="ExternalInput")
    ecb = nc.dram_tensor("ecb", [B * HPC, S, S], F16, kind="ExternalInput")
    ident = nc.dram_tensor("ident", [128, 64], F16, kind="ExternalInput")
    ones = nc.dram_tensor("ones", [1, 64], F16, kind="ExternalInput")
    outp = nc.dram_tensor("outp", [B, S, DM], F16, kind="ExternalOutput")

    with tile.TileContext(nc) as tc:
        with tc.tile_pool(name="consts", bufs=1) as consts, \
             tc.tile_pool(name="qkv", bufs=1) as qkv, \
             tc.tile_pool(name="xin", bufs=8) as xpool, \
             tc.tile_pool(name="cb", bufs=8) as cbpool, \
             tc.tile_pool(name="ex", bufs=3) as epool, \
             tc.tile_pool(name="misc", bufs=2) as mpool, \
             tc.tile_pool(name="ot", bufs=6) as otpool:

            id_sb = consts.tile([128, 64], F16, tag="id")
            nc.sync.dma_start(id_sb[:], ident[:])
            ones_sb = consts.tile([1, 64], F16, tag="ones")
            nc.sync.dma_start(ones_sb[:], ones[:])

            w_sbs = {}
            b_sbs = {}
            for nm, wdram, bdram in (("q", wq, bq), ("k", wk, bk), ("v", wv, bv)):
                w_sb = consts.tile([128, NDC * HD], F16, tag=f"w{nm}")
                # DRAM [chunk, dm128, hd] -> SBUF [dm128, chunk*HD]
                nc.sync.dma_start(w_sb[:], wdram[:].transpose([1, 0, 2]))
                b_sb = consts.tile([HD, 1], F32, tag=f"b{nm}")
                nc.sync.dma_start(b_sb[:], bdram[:])
                w_sbs[nm] = w_sb
                b_sbs[nm] = b_sb
            wo_sb = consts.tile([HD, DM], F16, tag="wo")
            nc.sync.dma_start(wo_sb[:], wo[:])

            # Persistent per-batch projection outputs.
            qt_sb = [qkv.tile([HD, S], F16, tag=f"qt{b}", name=f"qt{b}")
                     for b in range(B)]
            kt_sb = [qkv.tile([HD, S], F16, tag=f"kt{b}", name=f"kt{b}")
                     for b in range(B)]
            vt_sb = [qkv.tile([HD, S], F16, tag=f"vt{b}", name=f"vt{b}")
                     for b in range(B)]
            # V natural layout + ones column: per (head, kchunk) a [128, 65]
            # block at col (hl*NKC+kc)*65; col 64 stays 1.0 from the memset.
            vn_sb = [qkv.tile([128, HPC * NKC * 65], F16, tag=f"vn{b}", name=f"vn{b}")
                     for b in range(B)]
            ctx_sb = [qkv.tile([HD, S], F16, tag=f"ctx{b}", name=f"ctx{b}")
                      for b in range(B)]

            for b in range(B):
                nc.vector.memset(vn_sb[b][:], 1.0)

            # ---------------- projections ----------------
            with tc.tile_pool(name="proj_ps", bufs=5, space="PSUM") as ppool, \
                 tc.tile_pool(name="tp_ps", bufs=2, space="PSUM") as tpool:
                for b in range(B):
                    for nm, xdram, dest in (("q", xq, qt_sb), ("k", xk, kt_sb),
                                            ("v", xv, vt_sb)):
                        pss = [ppool.tile([HD, 512], F32, tag="proj", name=f"pp{b}{nm}{i}")
                               for i in range(4)]
                        for dc in range(NDC):
                            xt = xpool.tile([128, S], F16, tag="x")
                            nc.sync.dma_start(
                                xt[:], xdram[b, dc * 128:(dc + 1) * 128, :])
                            lhsT = w_sbs[nm][:, dc * HD:(dc + 1) * HD]
                            for sc in range(4):
                                nc.tensor.matmul(
                                    pss[sc][:], lhsT,
                                    xt[:, sc * 512:(sc + 1) * 512],
                                    start=(dc == 0), stop=(dc == NDC - 1))
                        for sc in range(4):
                            nc.scalar.activation(
                                dest[b][:, sc * 512:(sc + 1) * 512],
                                pss[sc][:], AF.Identity, bias=b_sbs[nm][:])

                    # V natural layout via PE transpose of Vt
                    for hl in range(HPC):
                        for kc in range(NKC):
                            tp = tpool.tile([128, 64], F16, tag="tp")
                            nc.tensor.transpose(
                                tp[:],
                                vt_sb[b][hl * 64:(hl + 1) * 64,
                                         kc * 128:(kc + 1) * 128],
                                id_sb[hl * 64:(hl + 1) * 64, :])
                            col = (hl * NKC + kc) * 65
                            nc.vector.tensor_copy(
                                vn_sb[b][:, col:col + 64], tp[:])

            # ------------- attention + interleaved out-proj -------------
            # Normalization of pair p is deferred into pair p+1's kc-loop so
            # the recip->bcast chain never stalls the PE stream; out-proj of
            # batch b is emitted inside the following pairs' loops so its
            # matmuls densify PE and its stores overlap attention DMA.
            with tc.tile_pool(name="sc_ps", bufs=2, space="PSUM") as spool, \
                 tc.tile_pool(name="cx_ps", bufs=2, space="PSUM") as cxpool:

                def emit_normalize(b, hl, qc, ctxu, invh, ni):
                    hbase = hl * 64
                    q0 = qc * QW
                    bc_ps = spool.tile([64, QW], F32, tag="sc",
                                       name=f"bcps_{ni}")
                    for nn in range(2):
                        nc.tensor.matmul(
                            bc_ps[:, nn * 512:(nn + 1) * 512],
                            ones_sb[:],
                            invh[:, nn * 512:(nn + 1) * 512],
                            start=True, stop=True)
                    bc_sb = mpool.tile([64, QW], F16, tag="bcs",
                                       name=f"bcsb_{ni}")
                    nc.scalar.copy(bc_sb[:], bc_ps[:])
                    nc.vector.tensor_mul(
                        ctx_sb[b][hbase:hbase + 64, q0:q0 + QW],
                        ctxu[:], bc_sb[:])

                def emit_outproj_block(b, si):
                    op_ps = cxpool.tile([128, DM], F32, tag="cx",
                                        name=f"op{b}_{si}")
                    lhsT = ctx_sb[b][:, si * 128:(si + 1) * 128]
                    for nn in range(2):
                        nc.tensor.matmul(
                            op_ps[:, nn * 512:(nn + 1) * 512],
                            lhsT, wo_sb[:, nn * 512:(nn + 1) * 512],
                            start=True, stop=True)
                    ot = otpool.tile([128, DM], F16, tag="ot")
                    nc.vector.tensor_copy(ot[:], op_ps[:])
                    nc.sync.dma_start(
                        outp[b, si * 128:(si + 1) * 128, :], ot[:])

                pairs = [(b, hl) for b in range(B) for hl in range(HPC)]
                pending_norm = None   # (b, hl, qc, ctxu, invh)
                pending_oproj = []    # list of (b, si)
                ni = 0
                for pi, (b, hl) in enumerate(pairs):
                    hbase = hl * 64
                    for qc in range(NQC):
                        q0 = qc * QW
                        cx = cxpool.tile([65, QW], F32, tag="cx",
                                         name=f"cx{pi}_{qc}")
                        for kc in range(NKC):
                            sc_ps = spool.tile([128, QW], F32, tag="sc")
                            klhs = kt_sb[b][hbase:hbase + 64,
                                            kc * 128:(kc + 1) * 128]
                            for nn in range(2):
                                nc.tensor.matmul(
                                    sc_ps[:, nn * 512:(nn + 1) * 512],
                                    klhs,
                                    qt_sb[b][hbase:hbase + 64,
                                             q0 + nn * 512:q0 + (nn + 1) * 512],
                                    start=True, stop=True)
                            er = epool.tile([128, QW], F16, tag="eraw")
                            nc.scalar.activation(er[:], sc_ps[:], AF.Exp)
                            cb = cbpool.tile([128, QW], F16, tag="cb")
                            nc.sync.dma_start(
                                cb[:],
                                ecb[b * HPC + hl,
                                    kc * 128:(kc + 1) * 128,
                                    q0:q0 + QW])
                            ex = epool.tile([128, QW], F16, tag="ex")
                            nc.gpsimd.tensor_mul(ex[:], er[:], cb[:])
                            vcol = (hl * NKC + kc) * 65
                            for nn in range(2):
                                nc.tensor.matmul(
                                    cx[:, nn * 512:(nn + 1) * 512],
                                    vn_sb[b][:, vcol:vcol + 65],
                                    ex[:, nn * 512:(nn + 1) * 512],
                                    start=(kc == 0), stop=(kc == NKC - 1))
                            if kc == 7 and pending_norm is not None:
                                emit_normalize(*pending_norm, ni)
                                ni += 1
                                pending_norm = None
                            if kc in (3, 7, 11, 15) and pending_oproj:
                                emit_outproj_block(*pending_oproj.pop(0))
                        # Evacuate cx eagerly: unnormalized ctx + inverse of
                        # the softmax denominator; frees the PSUM slot so the
                        # deferred normalize never blocks the "cx" pool.
                        ctxu = mpool.tile([64, QW], F32, tag="ctxu",
                                          name=f"ctxu{pi}_{qc}", bufs=3)
                        nc.vector.tensor_copy(ctxu[:], cx[0:64, :])
                        inv32 = mpool.tile([1, QW], F32, tag="inv32",
                                           name=f"inv32_{pi}_{qc}", bufs=3)
                        nc.vector.reciprocal(inv32[:], cx[64:65, :])
                        invh = mpool.tile([1, QW], F16, tag="invh",
                                          name=f"invh_{pi}_{qc}", bufs=3)
                        nc.scalar.copy(invh[:], inv32[:])
                        pending_norm = (b, hl, qc, ctxu, invh)
                    if hl == HPC - 1:
                        # batch b's ctx completes once its last normalize is
                        # emitted (during the next pair); queue its out-proj.
                        pending_oproj.extend((b, si) for si in range(NSB))
                emit_normalize(*pending_norm, ni)
                for b_si in pending_oproj:
                    emit_outproj_block(*b_si)
    return nc


_PROGRAM_CACHE = {}


def _get_program(split=True):
    key = ("nc", split)
    if key not in _PROGRAM_CACHE:
        nc = _build_program()
        if split:
            _split_multiwait_dmas(nc)
        _PROGRAM_CACHE[key] = nc
    return _PROGRAM_CACHE[key]


def _make_in_maps(query, key, value, mask, position_bias,
                  w_q, b_q, w_k, b_k, w_v, b_v, w_o, b_o):
    query = np.asarray(query, dtype=np.float32)
    key = np.asarray(key, dtype=np.float32)
    value = np.asarray(value, dtype=np.float32)
    mask = np.asarray(mask)
    position_bias = np.asarray(position_bias, dtype=np.float32)
    w_q = np.asarray(w_q, dtype=np.float32)
    b_q = np.asarray(b_q, dtype=np.float32)
    w_k = np.asarray(w_k, dtype=np.float32)
    b_k = np.asarray(b_k, dtype=np.float32)
    w_v = np.asarray(w_v, dtype=np.float32)
    b_v = np.asarray(b_v, dtype=np.float32)
    w_o = np.asarray(w_o, dtype=np.float32)
    b_o = np.asarray(b_o, dtype=np.float32)

    scale = 1.0 / np.sqrt(np.float32(DH))

    # x^T per batch: [B, DM, S] fp16
    xq_h = np.ascontiguousarray(query.transpose(0, 2, 1)).astype(np.float16)
    xk_h = np.ascontiguousarray(key.transpose(0, 2, 1)).astype(np.float16)
    xv_h = np.ascontiguousarray(value.transpose(0, 2, 1)).astype(np.float16)

    ident = np.concatenate([np.eye(64), np.eye(64)], axis=0).astype(np.float16)
    ones64 = np.ones((1, 64), dtype=np.float16)

    # ecb[(b,hl)] = (exp(position_bias[h]) * mask[b]).T  in fp16, per core
    maskf = mask[:, 0].astype(np.float16)          # [B, S, S]
    epb = np.exp(position_bias[0])                  # [NH, S, S] fp32

    in_maps = []
    for c in range(NCORES):
        rows = slice(c * HD, (c + 1) * HD)
        wq_c = np.ascontiguousarray(
            (w_q[rows].T * scale)).astype(np.float16).reshape(NDC, 128, HD)
        wk_c = np.ascontiguousarray(w_k[rows].T).astype(np.float16).reshape(
            NDC, 128, HD)
        wv_c = np.ascontiguousarray(w_v[rows].T).astype(np.float16).reshape(
            NDC, 128, HD)
        bq_c = (b_q[rows] * scale).reshape(HD, 1).astype(np.float32)
        bk_c = b_k[rows].reshape(HD, 1).astype(np.float32)
        bv_c = b_v[rows].reshape(HD, 1).astype(np.float32)
        wo_c = np.ascontiguousarray(w_o[:, rows].T).astype(np.float16)

        ecb_c = np.empty((B * HPC, S, S), dtype=np.float16)
        for b in range(B):
            for hl in range(HPC):
                h = c * HPC + hl
                ecb_c[b * HPC + hl] = (
                    epb[h].T.astype(np.float16) * maskf[b].T)

        in_maps.append({
            "xq": xq_h, "xk": xk_h, "xv": xv_h,
            "wq": wq_c, "wk": wk_c, "wv": wv_c,
            "bq": bq_c, "bk": bk_c, "bv": bv_c,
            "wo": wo_c, "ecb": ecb_c,
            "ident": ident, "ones": ones64,
        })
    return in_maps


def kernel(query, key, value, mask, position_bias,
           w_q, b_q, w_k, b_k, w_v, b_v, w_o, b_o):
    b_o = np.asarray(b_o, dtype=np.float32)
    in_maps = _make_in_maps(query, key, value, mask, position_bias,
                            w_q, b_q, w_k, b_k, w_v, b_v, w_o, b_o)

    nc = _get_program()
    kwargs = {}
    if os.environ.get("KBENCH_TRACE"):
        kwargs["trace"] = True
        if os.environ.get("KBENCH_TRACE_DIR"):
            kwargs["tmpdir"] = os.environ["KBENCH_TRACE_DIR"]
    res = run_bass_kernel_spmd(nc, in_maps, list(range(NCORES)), **kwargs)
    _PROGRAM_CACHE["last_results"] = res

    out = np.zeros((B, S, DM), dtype=np.float32)
    for c in range(NCORES):
        out += res.results[c]["outp"].astype(np.float32)
    out += b_o
    return out

